# revision 33
# baseline (speedup 1.0000x reference)
"""Trainium2 Bass kernel: 6-layer causal transformer binary classifier.

Data-parallel over batch: B=8 rows -> 8 NeuronCores, one full forward per core.
Activations kept transposed ([H, S], H on partitions). Optimizations over the
plain version:
  - LayerNorm gamma/mean folded into the following projection weights host-side
    (W'' = diag(g) W - colmean(diag(g) W)); the device LN reduces to
    xs = x * rstd, one DVE op per H-tile, removing the DVE chains that stall
    the PE. ln beta and the V bias are folded into downstream biases.
  - PSUM evacuations ride the ACT engine (Identity/Gelu/Copy with fused
    per-partition bias); x^2 for LN variance on ACT Square.
  - Causal diagonal-band column restriction on scores/AV/denominator matmuls.
  - Chunk-level software pipelining (stats of chunk c+1 emitted between
    projection matmuls of chunk c) to keep the PE warm.
All matmuls bf16 (fp8 tested: quantization noise exceeds the accuracy gate).
"""

import numpy as np
import ml_dtypes

import concourse.bass as bass
import concourse.mybir as mybir
import concourse.tile as tile
from concourse import bacc
from concourse.bass_utils import run_bass_kernel_spmd

F32 = mybir.dt.float32
F32R = mybir.dt.float32r
BF16 = mybir.dt.bfloat16
F8 = mybir.dt.float8e4
I32 = mybir.dt.int32
DRM = mybir.MatmulPerfMode.DoubleRow

L_FULL, B_FULL, S_FULL, H, V = 6, 8, 2048, 768, 32000
FF = 4 * H
EPS = 1e-5
P = 128
HT = H // P          # 6 H-tiles
FT = FF // P         # 24 FF-tiles

# ppp column layout (per-partition params, [128, 48] per layer)
_BQ, _BK, _BO, _B2, _B1 = 0, 6, 12, 18, 24
_PPPW = 24 + FT

AF = mybir.ActivationFunctionType
OP = mybir.AluOpType


def _chunks(total, width):
    out = []
    c = 0
    while c < total:
        w = min(width, total - c)
        out.append((c, w))
        c += w
    return out


SA = 0.125   # attnT fp8 scale: stores exp(s)*SA
SV = 8.0     # vn fp8 scale: stores v*SV


def build_nc(S, L, vscales, q_last=True):
    """vscales: per-layer pow2 quantization scale of the (folded) V weights."""
    NT = S // P
    nc = bacc.Bacc("TRN2")

    emb_x = nc.declare_dram_parameter("tok_emb", [V, H], F32, isOutput=False)
    pos_x = nc.declare_dram_parameter("pos", [S, H], F32, isOutput=False)
    ids_x = nc.declare_dram_parameter("ids32", [NT, P, 1], I32, isOutput=False)
    ident_x = nc.declare_dram_parameter("ident", [P, P], F32, isOutput=False)
    cmask_x = nc.declare_dram_parameter("cmask", [P, P], BF16, isOutput=False)
    wqk_x = nc.declare_dram_parameter("wqkb", [L, P, 2, HT, HT, P], BF16, isOutput=False)
    wv_x = nc.declare_dram_parameter("wv8", [L, P, HT // 2, 2, H], F8, isOutput=False)
    wo_x = nc.declare_dram_parameter("wob", [L, P, HT, HT, P], BF16, isOutput=False)
    w1_x = nc.declare_dram_parameter("w1b", [L, P, FT, HT, P], BF16, isOutput=False)
    w2_x = nc.declare_dram_parameter("w2b", [L, P, HT, FT, P], BF16, isOutput=False)
    ppp_x = nc.declare_dram_parameter("ppp", [L, P, _PPPW], F32, isOutput=False)
    fpp_x = nc.declare_dram_parameter("fpp", [P, 18], F32, isOutput=False)
    clsb_x = nc.declare_dram_parameter("clsb", [1, 1], F32, isOutput=False)
    out_x = nc.declare_dram_parameter("out", [1, 1], F32, isOutput=True)

    sc = 1.0 / float(np.sqrt(H))

    with tile.TileContext(nc) as tc:
        with tc.tile_pool(name="persist", bufs=1) as pp:
            ident = pp.tile([P, P], F32, tag="ident")
            nc.sync.dma_start(out=ident, in_=ident_x[:])
            cmask = pp.tile([P, P], BF16, tag="cmask")
            nc.sync.dma_start(out=cmask, in_=cmask_x[:])
            ones_f = pp.tile([P, P], F32, tag="ones_f")
            nc.vector.memset(ones_f, 1.0)
            ones_r = pp.tile([P, P], F32R, tag="ones_r")
            nc.vector.tensor_copy(ones_r, ones_f)
            ones_b = pp.tile([P, P], BF16, tag="ones_b")
            nc.vector.memset(ones_b, 1.0)
            eps_t = pp.tile([P, 1], F32, tag="eps")
            nc.vector.memset(eps_t, EPS)
            ones8 = pp.tile([P, 2, P], F8, tag="ones8")
            nc.vector.memset(ones8, 1.0)
            ln8n = pp.tile([P, 1], F32, tag="ln8n")
            nc.vector.memset(ln8n, float(np.log(SA)))
            fpp = pp.tile([P, 18], F32, tag="fpp")
            nc.sync.dma_start(out=fpp, in_=fpp_x[:])

            xT = pp.tile([P, HT, S], F32R, tag="xT")
            xR = xT.bitcast(F32)  # read view for DVE

            # ---- embedding: gather + pos, then PE-transpose into xT ----
            with tc.tile_pool(name="emb", bufs=1) as ep, \
                 tc.tile_pool(name="embp", bufs=1, space="PSUM") as epp:
                xns = []
                for tt in range(NT):
                    ids_t = ep.tile([P, 1], I32, tag="ids", bufs=4)
                    nc.sync.dma_start(out=ids_t, in_=ids_x[tt])
                    xn = ep.tile([P, H], F32, tag="xn", bufs=8)
                    nc.gpsimd.indirect_dma_start(
                        out=xn[:], out_offset=None, in_=emb_x[:],
                        in_offset=bass.IndirectOffsetOnAxis(ap=ids_t[:, :1], axis=0))
                    pos_t = ep.tile([P, H], F32, tag="pos", bufs=4)
                    nc.sync.dma_start(out=pos_t, in_=pos_x[tt * P:(tt + 1) * P, :])
                    nc.vector.tensor_add(xn, xn, pos_t)
                    xns.append(xn)
                    for c in range(HT):
                        trp = epp.tile([P, P], F32, tag="tr", bufs=4, space="PSUM")
                        nc.tensor.transpose(out=trp[:], in_=xn[:, c * P:(c + 1) * P],
                                            identity=ident[:])
                        nc.vector.tensor_copy(xT[:, c, tt * P:(tt + 1) * P], trp)

            # ---- LN stats -> xsb = x * rstd (bf16), per 512-chunk ----
            def emit_ln(sb, ps, c0, cw, xsb):
                s1 = ps.tile([P, 512], F32, tag="st", bufs=2, space="PSUM")
                for c in range(HT):
                    nc.tensor.matmul(s1[:, :cw], ones_r, xT[:, c, c0:c0 + cw],
                                     start=(c == 0), stop=(c == HT - 1))
                s2 = ps.tile([P, 512], F32, tag="st", bufs=2, space="PSUM")
                for c in range(HT):
                    sq = sb.tile([P, 512], BF16, tag="sq", bufs=2)
                    nc.scalar.activation(out=sq[:, :cw], in_=xR[:, c, c0:c0 + cw],
                                         func=AF.Square)
                    nc.tensor.matmul(s2[:, :cw], ones_b, sq[:, :cw],
                                     start=(c == 0), stop=(c == HT - 1))
                mn2 = sb.tile([P, 512], F32, tag="mn2", bufs=1)
                nc.scalar.activation(out=mn2[:, :cw], in_=s1[:, :cw],
                                     func=AF.Square, scale=1.0 / H)
                var = sb.tile([P, 512], F32, tag="var", bufs=1)
                nc.vector.scalar_tensor_tensor(
                    out=var[:, :cw], in0=s2[:, :cw], scalar=1.0 / H,
                    in1=mn2[:, :cw], op0=OP.mult, op1=OP.subtract)
                rstd = sb.tile([P, 512], F32, tag="rstd", bufs=2)
                nc.scalar.activation(out=rstd[:, :cw], in_=var[:, :cw],
                                     func=AF.Abs_reciprocal_sqrt, bias=eps_t[:])
                for c in range(HT):
                    nc.vector.tensor_mul(xsb[:, c, :cw],
                                         xR[:, c, c0:c0 + cw], rstd[:, :cw])

            # ---- transformer layers ----
            for l in range(L):
                last = q_last and (l == L - 1)

                # ===== attention =====
                with tc.tile_pool(name=f"at{l}", bufs=1) as sb:
                    ppp = sb.tile([P, _PPPW], F32, tag="ppp")
                    nc.sync.dma_start(out=ppp, in_=ppp_x[l])

                    qt = sb.tile([P, HT, S], BF16, tag="qt")
                    kt_ = sb.tile([P, HT, S], BF16, tag="kt")
                    vn = sb.tile([P, NT, H], F8, tag="vn")

                    wp = tc.alloc_tile_pool(name=f"atw{l}", bufs=1)
                    wqk = wp.tile([P, 2, HT, HT, P], BF16, tag="wqk")
                    nc.sync.dma_start(out=wqk, in_=wqk_x[l])
                    wv = wp.tile([P, HT // 2, 2, H], F8, tag="wv")
                    nc.sync.dma_start(out=wv, in_=wv_x[l])
                    v_evac = SV / vscales[l]

                    qkv_ps = tc.alloc_tile_pool(name=f"atp{l}", bufs=1, space="PSUM")
                    ps = qkv_ps

                    def emit_qkv(c0, cw, need_q, xsb, xs8):
                        for m in range(HT):
                            if need_q:
                                pj = ps.tile([P, 512], F32, tag="qp", bufs=2,
                                             space="PSUM")
                                for k in range(HT):
                                    nc.tensor.matmul(
                                        pj[:, :cw], wqk[:, 0, m, k, :], xsb[:, k, :cw],
                                        start=(k == 0), stop=(k == HT - 1))
                                nc.scalar.activation(
                                    out=qt[:, m, c0:c0 + cw], in_=pj[:, :cw],
                                    func=AF.Identity,
                                    bias=ppp[:, _BQ + m:_BQ + m + 1], scale=sc)
                            pk = ps.tile([P, 512], F32, tag="qp", bufs=2, space="PSUM")
                            for k in range(HT):
                                nc.tensor.matmul(
                                    pk[:, :cw], wqk[:, 1, m, k, :], xsb[:, k, :cw],
                                    start=(k == 0), stop=(k == HT - 1))
                            nc.vector.tensor_scalar_add(
                                kt_[:, m, c0:c0 + cw], pk[:, :cw],
                                ppp[:, _BK + m:_BK + m + 1])
                        for t in range(cw // P):
                            tt = (c0 // P) + t
                            pv = ps.tile([P, H], F32, tag="vp", bufs=2, space="PSUM")
                            for (j0, jw) in _chunks(H, 512):
                                for kp in range(HT // 2):
                                    nc.tensor.matmul(
                                        pv[:, j0:j0 + jw],
                                        xs8[:, 2 * kp:2 * kp + 2, t * P:(t + 1) * P],
                                        wv[:, kp, :, j0:j0 + jw],
                                        start=(kp == 0), stop=(kp == HT // 2 - 1),
                                        perf_mode=DRM)
                            nc.scalar.activation(out=vn[:, tt, :], in_=pv[:],
                                                 func=AF.Copy, scale=v_evac)

                    prev = None
                    for (c0, cw) in _chunks(S, 512):
                        xsb = sb.tile([P, HT, 512], BF16, tag="xsb", bufs=2)
                        emit_ln(sb, ps, c0, cw, xsb)
                        xs8 = sb.tile([P, HT, 512], F8, tag="xs8", bufs=2)
                        nc.vector.tensor_copy(xs8[:, :, :cw], xsb[:, :, :cw])
                        if prev is not None:
                            emit_qkv(*prev)
                        need_q = (not last) or (c0 + cw > S - P)
                        prev = (c0, cw, need_q, xsb, xs8)
                    emit_qkv(*prev)

                    qkv_ps.release()
                    wp.release()
                    wp2 = tc.alloc_tile_pool(name=f"atw2{l}", bufs=1)
                    wo = wp2.tile([P, HT, HT, P], BF16, tag="wo")
                    nc.sync.dma_start(out=wo, in_=wo_x[l])
                    att_ps = tc.alloc_tile_pool(name=f"atq{l}", bufs=1, space="PSUM")
                    ps = att_ps

                    q_blocks = [(S - P, P)] if last else _chunks(S, 512)

                    def kt_ranges(q0, qw):
                        ktmax = (q0 + qw - 1) // P
                        out = []
                        for kt in range(ktmax + 1):
                            lo = kt * P - q0
                            d0 = max(lo, 0)
                            d1 = min(lo + P, qw)
                            out.append((kt, lo, d0, d1))
                        return out, ktmax

                    def emit_block(q0, qw):
                        rng, ktmax = kt_ranges(q0, qw)
                        # kt pairs for DoubleRow AV/den; zero the gap where the
                        # odd tile starts later than the even one
                        npair = (ktmax + 1) // 2
                        pairs = []
                        for t in range(npair):
                            d0e = rng[2 * t][2]
                            d0o = rng[2 * t + 1][2]
                            pairs.append((t, d0e, d0o))
                        attnT = sb.tile([P, NT, 512], F8, tag="attnT", bufs=1)
                        for (kt, lo, d0, d1) in rng:
                            scp = ps.tile([P, 512], F32, tag="sc", bufs=2, space="PSUM")
                            for c in range(HT):
                                nc.tensor.matmul(
                                    scp[:, d0:qw], kt_[:, c, kt * P:(kt + 1) * P],
                                    qt[:, c, q0 + d0:q0 + qw],
                                    start=(c == 0), stop=(c == HT - 1))
                            if lo + P <= 0:
                                nc.scalar.activation(out=attnT[:, kt, :qw],
                                                     in_=scp[:, :qw], func=AF.Exp,
                                                     bias=ln8n[:])
                            else:
                                dt_ = sb.tile([P, P], F8, tag="dtmp", bufs=2)
                                nc.scalar.activation(out=dt_[:, :d1 - d0],
                                                     in_=scp[:, d0:d1], func=AF.Exp,
                                                     bias=ln8n[:])
                                nc.vector.tensor_mul(attnT[:, kt, d0:d1],
                                                     dt_[:, :d1 - d0],
                                                     cmask[:, d0 - lo:d1 - lo])
                                if d1 < qw:
                                    nc.scalar.activation(out=attnT[:, kt, d1:qw],
                                                         in_=scp[:, d1:qw],
                                                         func=AF.Exp, bias=ln8n[:])
                        for (t, d0e, d0o) in pairs:
                            if d0o > d0e:
                                nc.vector.memset(attnT[:, 2 * t + 1, d0e:d0o], 0.0)
                        dnp = ps.tile([P, 512], F32, tag="dn", bufs=2, space="PSUM")
                        for (t, d0e, d0o) in pairs:
                            nc.tensor.matmul(dnp[:, d0e:qw], ones8[:],
                                             attnT[:, 2 * t:2 * t + 2, d0e:qw],
                                             start=(t == 0), stop=(t == npair - 1),
                                             perf_mode=DRM)
                        # 1/x via exp(-ln(x)) on ACT; SA fold rides the bias
                        dnl = sb.tile([P, 512], F32, tag="dnl", bufs=1)
                        nc.scalar.activation(out=dnl[:, :qw], in_=dnp[:, :qw],
                                             func=AF.Ln)
                        dn = sb.tile([P, 512], F32, tag="dns", bufs=2)
                        nc.scalar.activation(out=dn[:, :qw], in_=dnl[:, :qw],
                                             func=AF.Exp, scale=-1.0, bias=ln8n[:])
                        ot = sb.tile([P, HT, 512], BF16, tag="ot")
                        for m in range(HT):
                            avp = ps.tile([P, 512], F32, tag="av", bufs=2, space="PSUM")
                            for (t, d0e, d0o) in pairs:
                                nc.tensor.matmul(
                                    avp[:, d0e:qw],
                                    vn[:, 2 * t:2 * t + 2, m * P:(m + 1) * P],
                                    attnT[:, 2 * t:2 * t + 2, d0e:qw],
                                    start=(t == 0), stop=(t == npair - 1),
                                    perf_mode=DRM)
                            nc.vector.tensor_mul(ot[:, m, :qw], avp[:, :qw],
                                                 dn[:, :qw])
                        for m2 in range(HT):
                            wop = ps.tile([P, 512], F32, tag="wo", bufs=2, space="PSUM")
                            for k in range(HT):
                                nc.tensor.matmul(
                                    wop[:, :qw], wo[:, m2, k, :], ot[:, k, :qw],
                                    start=(k == 0), stop=(k == HT - 1))
                            otmp = sb.tile([P, 512], BF16, tag="otmp", bufs=2)
                            nc.scalar.activation(out=otmp[:, :qw], in_=wop[:, :qw],
                                                 func=AF.Identity,
                                                 bias=ppp[:, _BO + m2:_BO + m2 + 1])
                            nc.vector.tensor_add(xT[:, m2, q0:q0 + qw],
                                                 otmp[:, :qw], xR[:, m2, q0:q0 + qw])

                    for (q0, qw) in q_blocks:
                        emit_block(q0, qw)
                    att_ps.release()
                    wp2.release()

                # ===== FFN =====
                with tc.tile_pool(name=f"ff{l}", bufs=1) as sb, \
                     tc.tile_pool(name=f"ffp{l}", bufs=1, space="PSUM") as ps:
                    ppp = sb.tile([P, _PPPW], F32, tag="ppp")
                    nc.sync.dma_start(out=ppp, in_=ppp_x[l])
                    w1 = sb.tile([P, FT, HT, P], BF16, tag="w1")
                    nc.sync.dma_start(out=w1, in_=w1_x[l])
                    w2 = sb.tile([P, HT, FT, P], BF16, tag="w2")
                    nc.sync.dma_start(out=w2, in_=w2_x[l])
                    f_chunks = [(S - P, P)] if last else _chunks(S, 512)

                    def emit_ffn(c0, cw, xsb):
                        g1 = sb.tile([P, FT, 512], BF16, tag="g1", bufs=1)
                        for m in range(FT):
                            f1p = ps.tile([P, 512], F32, tag="fp", bufs=4,
                                          space="PSUM")
                            for k in range(HT):
                                nc.tensor.matmul(
                                    f1p[:, :cw], w1[:, m, k, :], xsb[:, k, :cw],
                                    start=(k == 0), stop=(k == HT - 1))
                            nc.scalar.activation(out=g1[:, m, :cw], in_=f1p[:, :cw],
                                                 func=AF.Gelu,
                                                 bias=ppp[:, _B1 + m:_B1 + m + 1])
                        for m2 in range(HT):
                            f2p = ps.tile([P, 512], F32, tag="fp", bufs=4,
                                          space="PSUM")
                            for k in range(FT):
                                nc.tensor.matmul(
                                    f2p[:, :cw], w2[:, m2, k, :], g1[:, k, :cw],
                                    start=(k == 0), stop=(k == FT - 1))
                            ftmp = sb.tile([P, 512], BF16, tag="ftmp", bufs=2)
                            nc.scalar.activation(out=ftmp[:, :cw], in_=f2p[:, :cw],
                                                 func=AF.Identity,
                                                 bias=ppp[:, _B2 + m2:_B2 + m2 + 1])
                            nc.vector.tensor_add(xT[:, m2, c0:c0 + cw],
                                                 ftmp[:, :cw], xR[:, m2, c0:c0 + cw])

                    prevf = None
                    for (c0, cw) in f_chunks:
                        xsb = sb.tile([P, HT, 512], BF16, tag="xsb", bufs=2)
                        emit_ln(sb, ps, c0, cw, xsb)
                        if prevf is not None:
                            emit_ffn(*prevf)
                        prevf = (c0, cw, xsb)
                    emit_ffn(*prevf)

            # ---- final LN on last column + classifier ----
            with tc.tile_pool(name="head", bufs=1) as sb, \
                 tc.tile_pool(name="headp", bufs=1, space="PSUM") as ps:
                col = S - 1
                s1 = ps.tile([P, 1], F32, tag="hp", bufs=2, space="PSUM")
                for c in range(HT):
                    nc.tensor.matmul(s1, ones_f, xR[:, c, col:col + 1],
                                     start=(c == 0), stop=(c == HT - 1))
                sqc = sb.tile([P, HT, 1], F32, tag="hsq")
                nc.vector.tensor_mul(sqc, xR[:, :, col:col + 1], xR[:, :, col:col + 1])
                s2 = ps.tile([P, 1], F32, tag="hp", bufs=2, space="PSUM")
                for c in range(HT):
                    nc.tensor.matmul(s2, ones_f, sqc[:, c, :],
                                     start=(c == 0), stop=(c == HT - 1))
                mean = sb.tile([P, 1], F32, tag="hmean")
                nc.vector.tensor_scalar_mul(mean, s1, 1.0 / H)
                rstd = sb.tile([P, 1], F32, tag="hrstd")
                nc.vector.tensor_scalar_mul(rstd, s2, 1.0 / H)
                m2_ = sb.tile([P, 1], F32, tag="hm2")
                nc.vector.tensor_mul(m2_, mean, mean)
                nc.vector.tensor_sub(rstd, rstd, m2_)
                nc.scalar.activation(out=rstd, in_=rstd, func=AF.Sqrt, bias=eps_t[:])
                nc.vector.reciprocal(rstd, rstd)
                nf = sb.tile([P, HT, 1], F32, tag="hnf")
                for c in range(HT):
                    t = sb.tile([P, 1], F32, tag="ht", bufs=2)
                    nc.vector.tensor_sub(t, xR[:, c, col:col + 1], mean)
                    nc.vector.tensor_mul(t, t, rstd)
                    nc.vector.tensor_scalar(
                        out=nf[:, c, :], in0=t,
                        scalar1=fpp[:, c:c + 1], scalar2=fpp[:, 6 + c:7 + c],
                        op0=OP.mult, op1=OP.add)
                lp = ps.tile([P, 1], F32, tag="hp", bufs=2, space="PSUM")
                for c in range(HT):
                    nc.tensor.matmul(lp[0:1, :], nf[:, c, :], fpp[:, 12 + c:13 + c],
                                     start=(c == 0), stop=(c == HT - 1))
                cb = sb.tile([P, 1], F32, tag="hcb")
                nc.sync.dma_start(out=cb[0:1, :], in_=clsb_x[:])
                oo = sb.tile([P, 1], F32, tag="hoo")
                nc.vector.tensor_add(oo[0:1, :], lp[0:1, :], cb[0:1, :])
                nc.sync.dma_start(out=out_x[:], in_=oo[0:1, :])

    nc.finalize()
    return nc


def _pack_host(inputs, S, L):
    """Shared (replicated) host-side packed arrays with LN folds."""
    f32 = np.float32
    bf16 = ml_dtypes.bfloat16

    def npf(x):
        return np.asarray(x, dtype=f32)

    Wq, Wk, Wv = npf(inputs["Wq"]), npf(inputs["Wk"]), npf(inputs["Wv"])
    Wo, W1, W2 = npf(inputs["Wo"]), npf(inputs["W1"]), npf(inputs["W2"])
    bq, bk, bv, bo = (npf(inputs["bq"]), npf(inputs["bk"]),
                      npf(inputs["bv"]), npf(inputs["bo"]))
    b1, b2 = npf(inputs["b1"]), npf(inputs["b2"])
    g1f, b1f = npf(inputs["ln1_g"]), npf(inputs["ln1_b"])
    g2f, b2f = npf(inputs["ln2_g"]), npf(inputs["ln2_b"])
    sc = 1.0 / float(np.sqrt(H))

    f8 = ml_dtypes.float8_e4m3
    wqkb = np.empty((L, P, 2, HT, HT, P), dtype=bf16)
    wv8 = np.empty((L, P, HT // 2, 2, H), dtype=f8)
    vscales = []
    wob = np.empty((L, P, HT, HT, P), dtype=bf16)
    w1b = np.empty((L, P, FT, HT, P), dtype=bf16)
    w2b = np.empty((L, P, HT, FT, P), dtype=bf16)
    ppp = np.zeros((L, P, _PPPW), dtype=f32)

    def stat(w, mt):  # [K, mt*128] -> [P(part of K), mt, K//128, 128]
        kt = w.shape[0] // P
        return np.ascontiguousarray(
            w.reshape(kt, P, mt, P).transpose(1, 2, 0, 3))

    def pcol(v, n):  # [n*128] -> [128, n]
        return v.reshape(n, P).T

    for l in range(L):
        Aq = Wq[l] * g1f[l][:, None]
        Ak = Wk[l] * g1f[l][:, None]
        Av = Wv[l] * g1f[l][:, None]
        Wqd = Aq - Aq.mean(0, keepdims=True)
        Wkd = Ak - Ak.mean(0, keepdims=True)
        Wvd = Av - Av.mean(0, keepdims=True)
        bqp = bq[l] + b1f[l] @ Wq[l]
        bkp = bk[l] + b1f[l] @ Wk[l]
        bvp = bv[l] + b1f[l] @ Wv[l]
        bop = bo[l] + bvp @ Wo[l]      # v bias moved past attention (rows sum to 1)
        A1 = W1[l] * g2f[l][:, None]
        W1d = A1 - A1.mean(0, keepdims=True)
        b1p = b1[l] + b2f[l] @ W1[l]

        wqkb[l, :, 0] = stat(Wqd, HT).astype(bf16)
        wqkb[l, :, 1] = stat(Wkd, HT).astype(bf16)
        sv_ = float(2.0 ** np.floor(np.log2(240.0 / np.abs(Wvd).max() / 2.0)))
        vscales.append(sv_)
        wv8q = (Wvd * sv_).reshape(HT // 2, 2, P, H).transpose(2, 0, 1, 3)
        assert np.isfinite(wv8q.astype(f8).astype(np.float32)).all()
        wv8[l] = np.ascontiguousarray(wv8q).astype(f8)
        wob[l] = stat(Wo[l], HT).astype(bf16)
        w1b[l] = stat(W1d, FT).astype(bf16)
        w2b[l] = stat(W2[l], HT).astype(bf16)

        ppp[l, :, _BQ:_BQ + HT] = pcol(bqp * sc, HT)
        ppp[l, :, _BK:_BK + HT] = pcol(bkp, HT)
        ppp[l, :, _BO:_BO + HT] = pcol(bop, HT)
        ppp[l, :, _B2:_B2 + HT] = pcol(b2[l], HT)
        ppp[l, :, _B1:_B1 + FT] = pcol(b1p, FT)

    fpp = np.concatenate([
        npf(inputs["fln_g"]).reshape(HT, P).T,
        npf(inputs["fln_b"]).reshape(HT, P).T,
        npf(inputs["cls_W"]).reshape(HT, P).T,
    ], axis=1)

    cm = (np.arange(P)[None, :] >= np.arange(P)[:, None])  # [k, q] valid q>=k

    shared = {
        "tok_emb": npf(inputs["tok_emb"]),
        "pos": npf(inputs["pos_emb"])[:S],
        "ident": np.eye(P, dtype=f32),
        "cmask": cm.astype(bf16),
        "wqkb": wqkb, "wv8": wv8, "wob": wob, "w1b": w1b, "w2b": w2b,
        "ppp": ppp,
        "fpp": np.ascontiguousarray(fpp),
        "clsb": npf(inputs["cls_b"]).reshape(1, 1),
    }
    return shared, vscales


_NC_CACHE = {}


def run_model(inputs, S=S_FULL, L=L_FULL, B=B_FULL, q_last=True, trace=False):
    mask = np.asarray(inputs["attention_mask"])
    if not np.all(mask == 1):
        raise NotImplementedError("padded attention_mask not supported")

    shared, vscales = _pack_host(inputs, S, L)
    ids = np.asarray(inputs["input_ids"]).astype(np.int32)  # [B, S]
    in_maps = []
    for b in range(B):
        m = dict(shared)
        m["ids32"] = np.ascontiguousarray(ids[b].reshape(S // P, P, 1))
        in_maps.append(m)

    key = (S, L, q_last, tuple(vscales))
    if key not in _NC_CACHE:
        _NC_CACHE[key] = build_nc(S, L, vscales, q_last)
    nc = _NC_CACHE[key]

    res = run_bass_kernel_spmd(nc, in_maps, list(range(B)), trace=trace)
    out = np.stack([res.results[b]["out"].reshape(1) for b in range(B)], axis=0)
    return out.astype(np.float32), res


def kernel(**inputs) -> np.ndarray:
    out, _ = run_model(inputs, S=S_FULL, L=L_FULL, B=B_FULL)
    return out


# revision 45
# speedup vs baseline: 1.1132x; 1.1132x over previous
"""Trainium2 Bass kernel: 6-layer causal transformer binary classifier.

Data-parallel over batch: B=8 rows -> 8 NeuronCores, one full forward per core.
Activations kept transposed ([H, S], H on partitions). Optimizations over the
plain version:
  - LayerNorm gamma/mean folded into the following projection weights host-side
    (W'' = diag(g) W - colmean(diag(g) W)); the device LN reduces to
    xs = x * rstd, one DVE op per H-tile, removing the DVE chains that stall
    the PE. ln beta and the V bias are folded into downstream biases.
  - PSUM evacuations ride the ACT engine (Identity/Gelu/Copy with fused
    per-partition bias); x^2 for LN variance on ACT Square.
  - Causal diagonal-band column restriction on scores/AV/denominator matmuls.
  - Chunk-level software pipelining (stats of chunk c+1 emitted between
    projection matmuls of chunk c) to keep the PE warm.
All matmuls bf16 (fp8 tested: quantization noise exceeds the accuracy gate).
"""

import numpy as np
import ml_dtypes

import concourse.bass as bass
import concourse.mybir as mybir
import concourse.tile as tile
from concourse import bacc
from concourse.bass_utils import run_bass_kernel_spmd

F32 = mybir.dt.float32
F32R = mybir.dt.float32r
BF16 = mybir.dt.bfloat16
F8 = mybir.dt.float8e4
I32 = mybir.dt.int32
DRM = mybir.MatmulPerfMode.DoubleRow

L_FULL, B_FULL, S_FULL, H, V = 6, 8, 2048, 768, 32000
FF = 4 * H
EPS = 1e-5
P = 128
HT = H // P          # 6 H-tiles
FT = FF // P         # 24 FF-tiles

# ppp column layout (per-partition params, [128, 48] per layer)
_BQ, _BK, _BO, _B2, _B1 = 0, 6, 12, 18, 24
_PPPW = 24 + FT

AF = mybir.ActivationFunctionType
OP = mybir.AluOpType


def _chunks(total, width):
    out = []
    c = 0
    while c < total:
        w = min(width, total - c)
        out.append((c, w))
        c += w
    return out


def build_nc(S, L, q_last=True):
    NT = S // P
    nc = bacc.Bacc("TRN2")

    emb_x = nc.declare_dram_parameter("tok_emb", [V, H], F32, isOutput=False)
    pos_x = nc.declare_dram_parameter("pos", [S, H], F32, isOutput=False)
    ids_x = nc.declare_dram_parameter("ids32", [NT, P, 1], I32, isOutput=False)
    ident_x = nc.declare_dram_parameter("ident", [P, P], F32, isOutput=False)
    cmask_x = nc.declare_dram_parameter("cmask", [P, P], BF16, isOutput=False)
    wqk_x = nc.declare_dram_parameter("wqkb", [L, P, 2, HT, HT, P], BF16, isOutput=False)
    wv_x = nc.declare_dram_parameter("wvb", [L, P, HT, H], BF16, isOutput=False)
    wo_x = nc.declare_dram_parameter("wob", [L, P, HT, HT, P], BF16, isOutput=False)
    w1_x = nc.declare_dram_parameter("w1b", [L, P, FT, HT, P], BF16, isOutput=False)
    w2_x = nc.declare_dram_parameter("w2b", [L, P, HT, FT, P], BF16, isOutput=False)
    ppp_x = nc.declare_dram_parameter("ppp", [L, P, _PPPW], F32, isOutput=False)
    fpp_x = nc.declare_dram_parameter("fpp", [P, 18], F32, isOutput=False)
    clsb_x = nc.declare_dram_parameter("clsb", [1, 1], F32, isOutput=False)
    out_x = nc.declare_dram_parameter("out", [1, 1], F32, isOutput=True)

    sc = 1.0 / float(np.sqrt(H))

    with tile.TileContext(nc) as tc:
        with tc.tile_pool(name="persist", bufs=1) as pp:
            ident = pp.tile([P, P], F32, tag="ident")
            nc.sync.dma_start(out=ident, in_=ident_x[:])
            cmask = pp.tile([P, P], BF16, tag="cmask")
            nc.sync.dma_start(out=cmask, in_=cmask_x[:])
            ones_f = pp.tile([P, P], F32, tag="ones_f")
            nc.vector.memset(ones_f, 1.0)
            ones_r = pp.tile([P, P], F32R, tag="ones_r")
            nc.vector.tensor_copy(ones_r, ones_f)
            ones_b = pp.tile([P, P], BF16, tag="ones_b")
            nc.vector.memset(ones_b, 1.0)
            eps_t = pp.tile([P, 1], F32, tag="eps")
            nc.vector.memset(eps_t, EPS)
            fpp = pp.tile([P, 18], F32, tag="fpp")
            nc.sync.dma_start(out=fpp, in_=fpp_x[:])

            xT = pp.tile([P, HT, S], F32R, tag="xT")
            xR = xT.bitcast(F32)  # read view for DVE

            # ---- embedding: gather + pos, then PE-transpose into xT ----
            with tc.tile_pool(name="emb", bufs=1) as ep, \
                 tc.tile_pool(name="embp", bufs=1, space="PSUM") as epp:
                xns = []
                for tt in range(NT):
                    ids_t = ep.tile([P, 1], I32, tag="ids", bufs=4)
                    nc.sync.dma_start(out=ids_t, in_=ids_x[tt])
                    xn = ep.tile([P, H], F32, tag="xn", bufs=8)
                    nc.gpsimd.indirect_dma_start(
                        out=xn[:], out_offset=None, in_=emb_x[:],
                        in_offset=bass.IndirectOffsetOnAxis(ap=ids_t[:, :1], axis=0))
                    pos_t = ep.tile([P, H], F32, tag="pos", bufs=4)
                    nc.sync.dma_start(out=pos_t, in_=pos_x[tt * P:(tt + 1) * P, :])
                    nc.vector.tensor_add(xn, xn, pos_t)
                    xns.append(xn)
                    for c in range(HT):
                        trp = epp.tile([P, P], F32, tag="tr", bufs=4, space="PSUM")
                        nc.tensor.transpose(out=trp[:], in_=xn[:, c * P:(c + 1) * P],
                                            identity=ident[:])
                        nc.vector.tensor_copy(xT[:, c, tt * P:(tt + 1) * P], trp)

            # ---- LN stats -> xsb = x * rstd (bf16), per 512-chunk ----
            def emit_ln(sb, ps, c0, cw, xsb):
                s1 = ps.tile([P, 512], F32, tag="st", bufs=2, space="PSUM")
                for c in range(HT):
                    nc.tensor.matmul(s1[:, :cw], ones_r, xT[:, c, c0:c0 + cw],
                                     start=(c == 0), stop=(c == HT - 1))
                s2 = ps.tile([P, 512], F32, tag="st", bufs=2, space="PSUM")
                for c in range(HT):
                    sq = sb.tile([P, 512], BF16, tag="sq", bufs=2)
                    nc.scalar.activation(out=sq[:, :cw], in_=xR[:, c, c0:c0 + cw],
                                         func=AF.Square)
                    nc.tensor.matmul(s2[:, :cw], ones_b, sq[:, :cw],
                                     start=(c == 0), stop=(c == HT - 1))
                mn2 = sb.tile([P, 512], F32, tag="mn2", bufs=1)
                nc.scalar.activation(out=mn2[:, :cw], in_=s1[:, :cw],
                                     func=AF.Square, scale=1.0 / H)
                var = sb.tile([P, 512], F32, tag="var", bufs=1)
                nc.vector.scalar_tensor_tensor(
                    out=var[:, :cw], in0=s2[:, :cw], scalar=1.0 / H,
                    in1=mn2[:, :cw], op0=OP.mult, op1=OP.subtract)
                rstd = sb.tile([P, 512], F32, tag="rstd", bufs=2)
                nc.scalar.activation(out=rstd[:, :cw], in_=var[:, :cw],
                                     func=AF.Abs_reciprocal_sqrt, bias=eps_t[:])
                for c in range(HT):
                    nc.vector.tensor_mul(xsb[:, c, :cw],
                                         xR[:, c, c0:c0 + cw], rstd[:, :cw])

            # ---- transformer layers ----
            for l in range(L):
                last = q_last and (l == L - 1)

                # ===== attention =====
                with tc.tile_pool(name=f"at{l}", bufs=1) as sb:
                    ppp = sb.tile([P, _PPPW], F32, tag="ppp")
                    nc.sync.dma_start(out=ppp, in_=ppp_x[l])

                    qt = sb.tile([P, HT, S], BF16, tag="qt")
                    kt_ = sb.tile([P, HT, S], BF16, tag="kt")
                    vn = sb.tile([P, NT, H], BF16, tag="vn")

                    wp = tc.alloc_tile_pool(name=f"atw{l}", bufs=1)
                    wqk = wp.tile([P, 2, HT, HT, P], BF16, tag="wqk")
                    nc.sync.dma_start(out=wqk, in_=wqk_x[l])
                    wv = wp.tile([P, HT, H], BF16, tag="wv")
                    nc.sync.dma_start(out=wv, in_=wv_x[l])

                    qkv_ps = tc.alloc_tile_pool(name=f"atp{l}", bufs=1, space="PSUM")
                    ps = qkv_ps

                    def emit_qkv(c0, cw, need_q, xsb):
                        for m in range(HT):
                            if need_q:
                                pj = ps.tile([P, 512], F32, tag="qp", bufs=2,
                                             space="PSUM")
                                for k in range(HT):
                                    nc.tensor.matmul(
                                        pj[:, :cw], wqk[:, 0, m, k, :], xsb[:, k, :cw],
                                        start=(k == 0), stop=(k == HT - 1))
                                nc.scalar.activation(
                                    out=qt[:, m, c0:c0 + cw], in_=pj[:, :cw],
                                    func=AF.Identity,
                                    bias=ppp[:, _BQ + m:_BQ + m + 1], scale=sc)
                            pk = ps.tile([P, 512], F32, tag="qp", bufs=2, space="PSUM")
                            for k in range(HT):
                                nc.tensor.matmul(
                                    pk[:, :cw], wqk[:, 1, m, k, :], xsb[:, k, :cw],
                                    start=(k == 0), stop=(k == HT - 1))
                            nc.vector.tensor_scalar_add(
                                kt_[:, m, c0:c0 + cw], pk[:, :cw],
                                ppp[:, _BK + m:_BK + m + 1])
                        for t in range(cw // P):
                            tt = (c0 // P) + t
                            pv = ps.tile([P, H], F32, tag="vp", bufs=2, space="PSUM")
                            for (j0, jw) in _chunks(H, 512):
                                for k in range(HT):
                                    nc.tensor.matmul(
                                        pv[:, j0:j0 + jw], xsb[:, k, t * P:(t + 1) * P],
                                        wv[:, k, j0:j0 + jw],
                                        start=(k == 0), stop=(k == HT - 1))
                            nc.scalar.activation(out=vn[:, tt, :], in_=pv[:],
                                                 func=AF.Copy)

                    prev = None
                    for (c0, cw) in _chunks(S, 512):
                        xsb = sb.tile([P, HT, 512], BF16, tag="xsb", bufs=2)
                        emit_ln(sb, ps, c0, cw, xsb)
                        if prev is not None:
                            emit_qkv(*prev)
                        need_q = (not last) or (c0 + cw > S - P)
                        prev = (c0, cw, need_q, xsb)
                    emit_qkv(*prev)

                    qkv_ps.release()
                    wp.release()
                    wp2 = tc.alloc_tile_pool(name=f"atw2{l}", bufs=1)
                    wo = wp2.tile([P, HT, HT, P], BF16, tag="wo")
                    nc.sync.dma_start(out=wo, in_=wo_x[l])
                    att_ps = tc.alloc_tile_pool(name=f"atq{l}", bufs=1, space="PSUM")
                    ps = att_ps

                    q_blocks = [(S - P, P)] if last else _chunks(S, 512)

                    def kt_ranges(q0, qw):
                        ktmax = (q0 + qw - 1) // P
                        out = []
                        for kt in range(ktmax + 1):
                            lo = kt * P - q0
                            d0 = max(lo, 0)
                            d1 = min(lo + P, qw)
                            out.append((kt, lo, d0, d1))
                        return out, ktmax

                    def emit_block(q0, qw):
                        rng, ktmax = kt_ranges(q0, qw)
                        attnT = sb.tile([P, NT, 512], BF16, tag="attnT", bufs=1)
                        for (kt, lo, d0, d1) in rng:
                            scp = ps.tile([P, 512], F32, tag="sc", bufs=2, space="PSUM")
                            for c in range(HT):
                                nc.tensor.matmul(
                                    scp[:, d0:qw], kt_[:, c, kt * P:(kt + 1) * P],
                                    qt[:, c, q0 + d0:q0 + qw],
                                    start=(c == 0), stop=(c == HT - 1))
                            if lo + P <= 0:
                                nc.scalar.activation(out=attnT[:, kt, :qw],
                                                     in_=scp[:, :qw], func=AF.Exp)
                            else:
                                dt_ = sb.tile([P, P], BF16, tag="dtmp", bufs=2)
                                nc.scalar.activation(out=dt_[:, :d1 - d0],
                                                     in_=scp[:, d0:d1], func=AF.Exp)
                                nc.vector.tensor_mul(attnT[:, kt, d0:d1],
                                                     dt_[:, :d1 - d0],
                                                     cmask[:, d0 - lo:d1 - lo])
                                if d1 < qw:
                                    nc.scalar.activation(out=attnT[:, kt, d1:qw],
                                                         in_=scp[:, d1:qw],
                                                         func=AF.Exp)
                        dnp = ps.tile([P, 512], F32, tag="dn", bufs=2, space="PSUM")
                        for (kt, lo, d0, d1) in rng:
                            nc.tensor.matmul(dnp[:, d0:qw], ones_b,
                                             attnT[:, kt, d0:qw],
                                             start=(kt == 0), stop=(kt == ktmax))
                        # 1/x via exp(-ln(x)) on ACT
                        dnl = sb.tile([P, 512], F32, tag="dnl", bufs=1)
                        nc.scalar.activation(out=dnl[:, :qw], in_=dnp[:, :qw],
                                             func=AF.Ln)
                        dn = sb.tile([P, 512], F32, tag="dns", bufs=2)
                        nc.scalar.activation(out=dn[:, :qw], in_=dnl[:, :qw],
                                             func=AF.Exp, scale=-1.0)
                        ot = sb.tile([P, HT, 512], BF16, tag="ot")
                        for m in range(HT):
                            avp = ps.tile([P, 512], F32, tag="av", bufs=2, space="PSUM")
                            for (kt, lo, d0, d1) in rng:
                                nc.tensor.matmul(
                                    avp[:, d0:qw], vn[:, kt, m * P:(m + 1) * P],
                                    attnT[:, kt, d0:qw],
                                    start=(kt == 0), stop=(kt == ktmax))
                            nc.vector.tensor_mul(ot[:, m, :qw], avp[:, :qw],
                                                 dn[:, :qw])
                        for m2 in range(HT):
                            wop = ps.tile([P, 512], F32, tag="wo", bufs=2, space="PSUM")
                            for k in range(HT):
                                nc.tensor.matmul(
                                    wop[:, :qw], wo[:, m2, k, :], ot[:, k, :qw],
                                    start=(k == 0), stop=(k == HT - 1))
                            otmp = sb.tile([P, 512], BF16, tag="otmp", bufs=2)
                            nc.scalar.activation(out=otmp[:, :qw], in_=wop[:, :qw],
                                                 func=AF.Identity,
                                                 bias=ppp[:, _BO + m2:_BO + m2 + 1])
                            nc.vector.tensor_add(xT[:, m2, q0:q0 + qw],
                                                 otmp[:, :qw], xR[:, m2, q0:q0 + qw])

                    for (q0, qw) in q_blocks:
                        emit_block(q0, qw)
                    att_ps.release()
                    wp2.release()

                # ===== FFN =====
                with tc.tile_pool(name=f"ff{l}", bufs=1) as sb, \
                     tc.tile_pool(name=f"ffp{l}", bufs=1, space="PSUM") as ps:
                    ppp = sb.tile([P, _PPPW], F32, tag="ppp")
                    nc.sync.dma_start(out=ppp, in_=ppp_x[l])
                    w1 = sb.tile([P, FT, HT, P], BF16, tag="w1")
                    nc.sync.dma_start(out=w1, in_=w1_x[l])
                    w2 = sb.tile([P, HT, FT, P], BF16, tag="w2")
                    nc.sync.dma_start(out=w2, in_=w2_x[l])
                    f_chunks = [(S - P, P)] if last else _chunks(S, 512)

                    def emit_ffn(c0, cw, xsb):
                        g1 = sb.tile([P, FT, 512], BF16, tag="g1", bufs=1)
                        for m in range(FT):
                            f1p = ps.tile([P, 512], F32, tag="fp", bufs=4,
                                          space="PSUM")
                            for k in range(HT):
                                nc.tensor.matmul(
                                    f1p[:, :cw], w1[:, m, k, :], xsb[:, k, :cw],
                                    start=(k == 0), stop=(k == HT - 1))
                            nc.scalar.activation(out=g1[:, m, :cw], in_=f1p[:, :cw],
                                                 func=AF.Gelu,
                                                 bias=ppp[:, _B1 + m:_B1 + m + 1])
                        for m2 in range(HT):
                            f2p = ps.tile([P, 512], F32, tag="fp", bufs=4,
                                          space="PSUM")
                            for k in range(FT):
                                nc.tensor.matmul(
                                    f2p[:, :cw], w2[:, m2, k, :], g1[:, k, :cw],
                                    start=(k == 0), stop=(k == FT - 1))
                            ftmp = sb.tile([P, 512], BF16, tag="ftmp", bufs=2)
                            nc.scalar.activation(out=ftmp[:, :cw], in_=f2p[:, :cw],
                                                 func=AF.Identity,
                                                 bias=ppp[:, _B2 + m2:_B2 + m2 + 1])
                            nc.vector.tensor_add(xT[:, m2, c0:c0 + cw],
                                                 ftmp[:, :cw], xR[:, m2, c0:c0 + cw])

                    prevf = None
                    for (c0, cw) in f_chunks:
                        xsb = sb.tile([P, HT, 512], BF16, tag="xsb", bufs=2)
                        emit_ln(sb, ps, c0, cw, xsb)
                        if prevf is not None:
                            emit_ffn(*prevf)
                        prevf = (c0, cw, xsb)
                    emit_ffn(*prevf)

            # ---- final LN on last column + classifier ----
            with tc.tile_pool(name="head", bufs=1) as sb, \
                 tc.tile_pool(name="headp", bufs=1, space="PSUM") as ps:
                col = S - 1
                s1 = ps.tile([P, 1], F32, tag="hp", bufs=2, space="PSUM")
                for c in range(HT):
                    nc.tensor.matmul(s1, ones_f, xR[:, c, col:col + 1],
                                     start=(c == 0), stop=(c == HT - 1))
                sqc = sb.tile([P, HT, 1], F32, tag="hsq")
                nc.vector.tensor_mul(sqc, xR[:, :, col:col + 1], xR[:, :, col:col + 1])
                s2 = ps.tile([P, 1], F32, tag="hp", bufs=2, space="PSUM")
                for c in range(HT):
                    nc.tensor.matmul(s2, ones_f, sqc[:, c, :],
                                     start=(c == 0), stop=(c == HT - 1))
                mean = sb.tile([P, 1], F32, tag="hmean")
                nc.vector.tensor_scalar_mul(mean, s1, 1.0 / H)
                rstd = sb.tile([P, 1], F32, tag="hrstd")
                nc.vector.tensor_scalar_mul(rstd, s2, 1.0 / H)
                m2_ = sb.tile([P, 1], F32, tag="hm2")
                nc.vector.tensor_mul(m2_, mean, mean)
                nc.vector.tensor_sub(rstd, rstd, m2_)
                nc.scalar.activation(out=rstd, in_=rstd, func=AF.Sqrt, bias=eps_t[:])
                nc.vector.reciprocal(rstd, rstd)
                nf = sb.tile([P, HT, 1], F32, tag="hnf")
                for c in range(HT):
                    t = sb.tile([P, 1], F32, tag="ht", bufs=2)
                    nc.vector.tensor_sub(t, xR[:, c, col:col + 1], mean)
                    nc.vector.tensor_mul(t, t, rstd)
                    nc.vector.tensor_scalar(
                        out=nf[:, c, :], in0=t,
                        scalar1=fpp[:, c:c + 1], scalar2=fpp[:, 6 + c:7 + c],
                        op0=OP.mult, op1=OP.add)
                lp = ps.tile([P, 1], F32, tag="hp", bufs=2, space="PSUM")
                for c in range(HT):
                    nc.tensor.matmul(lp[0:1, :], nf[:, c, :], fpp[:, 12 + c:13 + c],
                                     start=(c == 0), stop=(c == HT - 1))
                cb = sb.tile([P, 1], F32, tag="hcb")
                nc.sync.dma_start(out=cb[0:1, :], in_=clsb_x[:])
                oo = sb.tile([P, 1], F32, tag="hoo")
                nc.vector.tensor_add(oo[0:1, :], lp[0:1, :], cb[0:1, :])
                nc.sync.dma_start(out=out_x[:], in_=oo[0:1, :])

    nc.finalize()
    return nc


def _pack_host(inputs, S, L):
    """Shared (replicated) host-side packed arrays with LN folds."""
    f32 = np.float32
    bf16 = ml_dtypes.bfloat16

    def npf(x):
        return np.asarray(x, dtype=f32)

    Wq, Wk, Wv = npf(inputs["Wq"]), npf(inputs["Wk"]), npf(inputs["Wv"])
    Wo, W1, W2 = npf(inputs["Wo"]), npf(inputs["W1"]), npf(inputs["W2"])
    bq, bk, bv, bo = (npf(inputs["bq"]), npf(inputs["bk"]),
                      npf(inputs["bv"]), npf(inputs["bo"]))
    b1, b2 = npf(inputs["b1"]), npf(inputs["b2"])
    g1f, b1f = npf(inputs["ln1_g"]), npf(inputs["ln1_b"])
    g2f, b2f = npf(inputs["ln2_g"]), npf(inputs["ln2_b"])
    sc = 1.0 / float(np.sqrt(H))

    wqkb = np.empty((L, P, 2, HT, HT, P), dtype=bf16)
    wvb = np.empty((L, P, HT, H), dtype=bf16)
    wob = np.empty((L, P, HT, HT, P), dtype=bf16)
    w1b = np.empty((L, P, FT, HT, P), dtype=bf16)
    w2b = np.empty((L, P, HT, FT, P), dtype=bf16)
    ppp = np.zeros((L, P, _PPPW), dtype=f32)

    def stat(w, mt):  # [K, mt*128] -> [P(part of K), mt, K//128, 128]
        kt = w.shape[0] // P
        return np.ascontiguousarray(
            w.reshape(kt, P, mt, P).transpose(1, 2, 0, 3))

    def pcol(v, n):  # [n*128] -> [128, n]
        return v.reshape(n, P).T

    for l in range(L):
        Aq = Wq[l] * g1f[l][:, None]
        Ak = Wk[l] * g1f[l][:, None]
        Av = Wv[l] * g1f[l][:, None]
        Wqd = Aq - Aq.mean(0, keepdims=True)
        Wkd = Ak - Ak.mean(0, keepdims=True)
        Wvd = Av - Av.mean(0, keepdims=True)
        bqp = bq[l] + b1f[l] @ Wq[l]
        bkp = bk[l] + b1f[l] @ Wk[l]
        bvp = bv[l] + b1f[l] @ Wv[l]
        bop = bo[l] + bvp @ Wo[l]      # v bias moved past attention (rows sum to 1)
        A1 = W1[l] * g2f[l][:, None]
        W1d = A1 - A1.mean(0, keepdims=True)
        b1p = b1[l] + b2f[l] @ W1[l]

        wqkb[l, :, 0] = stat(Wqd, HT).astype(bf16)
        wqkb[l, :, 1] = stat(Wkd, HT).astype(bf16)
        wvb[l] = np.ascontiguousarray(
            Wvd.reshape(HT, P, H).transpose(1, 0, 2)).astype(bf16)
        wob[l] = stat(Wo[l], HT).astype(bf16)
        w1b[l] = stat(W1d, FT).astype(bf16)
        w2b[l] = stat(W2[l], HT).astype(bf16)

        ppp[l, :, _BQ:_BQ + HT] = pcol(bqp * sc, HT)
        ppp[l, :, _BK:_BK + HT] = pcol(bkp, HT)
        ppp[l, :, _BO:_BO + HT] = pcol(bop, HT)
        ppp[l, :, _B2:_B2 + HT] = pcol(b2[l], HT)
        ppp[l, :, _B1:_B1 + FT] = pcol(b1p, FT)

    fpp = np.concatenate([
        npf(inputs["fln_g"]).reshape(HT, P).T,
        npf(inputs["fln_b"]).reshape(HT, P).T,
        npf(inputs["cls_W"]).reshape(HT, P).T,
    ], axis=1)

    cm = (np.arange(P)[None, :] >= np.arange(P)[:, None])  # [k, q] valid q>=k

    shared = {
        "tok_emb": npf(inputs["tok_emb"]),
        "pos": npf(inputs["pos_emb"])[:S],
        "ident": np.eye(P, dtype=f32),
        "cmask": cm.astype(bf16),
        "wqkb": wqkb, "wvb": wvb, "wob": wob, "w1b": w1b, "w2b": w2b,
        "ppp": ppp,
        "fpp": np.ascontiguousarray(fpp),
        "clsb": npf(inputs["cls_b"]).reshape(1, 1),
    }
    return shared


_NC_CACHE = {}


def run_model(inputs, S=S_FULL, L=L_FULL, B=B_FULL, q_last=True, trace=False):
    mask = np.asarray(inputs["attention_mask"])
    if not np.all(mask == 1):
        raise NotImplementedError("padded attention_mask not supported")

    shared = _pack_host(inputs, S, L)
    ids = np.asarray(inputs["input_ids"]).astype(np.int32)  # [B, S]
    in_maps = []
    for b in range(B):
        m = dict(shared)
        m["ids32"] = np.ascontiguousarray(ids[b].reshape(S // P, P, 1))
        in_maps.append(m)

    key = (S, L, q_last)
    if key not in _NC_CACHE:
        _NC_CACHE[key] = build_nc(S, L, q_last)
    nc = _NC_CACHE[key]

    res = run_bass_kernel_spmd(nc, in_maps, list(range(B)), trace=trace)
    out = np.stack([res.results[b]["out"].reshape(1) for b in range(B)], axis=0)
    return out.astype(np.float32), res


def kernel(**inputs) -> np.ndarray:
    out, _ = run_model(inputs, S=S_FULL, L=L_FULL, B=B_FULL)
    return out
